# revision 4
# baseline (speedup 1.0000x reference)
"""LoRA layer (x @ W.T + (x@A)@B + bias) on 8 trn2 NeuronCores.

Data-parallel: core b computes batch b's (2048, 4096) output slice.
Per-core device work is a single fused matmul: the (4096-deep) base
projection accumulates 32 K=128 fp32r steps into PSUM, then one extra
K=17 step adds the low-rank correction and bias ((x@A | 1) @ (B ; bias)).
x@A (rank 16, 0.4% of FLOPs) and all transposes are host-side prep.
"""
import numpy as np

import concourse.mybir as mybir
import concourse.tile as tile
from concourse import bacc
from concourse.bass_utils import run_bass_kernel_spmd

BATCH, SEQ, DIN, DOUT, RANK = 8, 2048, 4096, 4096, 16
N_CORES = 8

KT = DIN // 128          # 32 contraction tiles
M_BLK = 1024             # tokens resident per block
N_MBLK = SEQ // M_BLK    # 2 blocks
MT_PER_BLK = M_BLK // 128  # 8 m-tiles -> 8 PSUM banks
OT = DOUT // 512         # 8 output-column tiles
F32R = mybir.dt.float32r
F32 = mybir.dt.float32

_nc_cache = []


def _build(no_x_dma=False, no_wt_dma=False, no_lora=False, no_out=False,
           wt_bufs=6, out_bufs=6, x_bufs=KT + 2, x_in_first_o=True, reps=1):
    nc = bacc.Bacc("TRN2", target_bir_lowering=False, debug=False)
    xT = nc.dram_tensor("xT", [DIN, SEQ], F32R, kind="ExternalInput")
    wT = nc.dram_tensor("wT", [DIN, DOUT], F32R, kind="ExternalInput")
    xaT = nc.dram_tensor("xaT", [RANK + 1, SEQ], F32R, kind="ExternalInput")
    bB = nc.dram_tensor("bB", [RANK + 1, DOUT], F32R, kind="ExternalInput")
    out = nc.dram_tensor("out", [SEQ, DOUT], F32, kind="ExternalOutput")

    with tile.TileContext(nc) as tc:
        with (
            tc.tile_pool(name="xblk", bufs=x_bufs) as xpool,
            tc.tile_pool(name="wt", bufs=wt_bufs) as wpool,
            tc.tile_pool(name="lora", bufs=1) as lpool,
            tc.tile_pool(name="outp", bufs=out_bufs) as opool,
            tc.tile_pool(name="psum", bufs=8, space="PSUM") as ppool,
        ):
            xa_sb = lpool.tile([RANK + 1, SEQ], F32R, tag="xa")
            nc.sync.dma_start(xa_sb[:], xaT[:, :])
            bB_sb = lpool.tile([RANK + 1, DOUT], F32R, tag="bB")
            nc.sync.dma_start(bB_sb[:], bB[:, :])

            def load_x(xpool, k, m0):
                xt = xpool.tile([128, M_BLK], F32R, name="x", tag="x")
                if no_x_dma:
                    nc.sync.dma_start(xt[:, :4], xT[k * 128:(k + 1) * 128, :4])
                else:
                    nc.sync.dma_start(
                        xt[:], xT[k * 128:(k + 1) * 128, m0:m0 + M_BLK])
                return xt

            import contextlib
            rep_ctx = tc.For_i(0, reps, 1) if reps > 1 else contextlib.nullcontext()
            with rep_ctx:
              for blk in range(N_MBLK):
                  m0 = blk * M_BLK
                  xtiles = [None] * KT
                  if not x_in_first_o:
                      for k in range(KT):
                          xtiles[k] = load_x(xpool, k, m0)
                  for o in range(OT):
                      o0 = o * 512
                      psums = [ppool.tile([128, 512], F32, name="ps", tag="ps")
                               for _ in range(MT_PER_BLK)]
                      for k in range(KT):
                          if xtiles[k] is None:
                              xtiles[k] = load_x(xpool, k, m0)
                          wt = wpool.tile([128, 512], F32R, name="w", tag="w")
                          if no_wt_dma:
                              nc.sync.dma_start(
                                  wt[:, :4], wT[k * 128:(k + 1) * 128, :4])
                          else:
                              nc.sync.dma_start(
                                  wt[:], wT[k * 128:(k + 1) * 128, o0:o0 + 512])
                          for mt in range(MT_PER_BLK):
                              nc.tensor.matmul(
                                  psums[mt][:],
                                  xtiles[k][:, mt * 128:(mt + 1) * 128],
                                  wt[:],
                                  start=(k == 0), stop=(no_lora and k == KT - 1))
                      for mt in range(MT_PER_BLK):
                          ms = m0 + mt * 128
                          if not no_lora:
                              nc.tensor.matmul(
                                  psums[mt][:],
                                  xa_sb[:, ms:ms + 128],
                                  bB_sb[:, o0:o0 + 512],
                                  start=False, stop=True)
                          ot = opool.tile([128, 512], F32, name="o", tag="o")
                          nc.vector.tensor_copy(ot[:], psums[mt][:])
                          if not no_out:
                              nc.sync.dma_start(
                                  out[ms:ms + 128, o0:o0 + 512], ot[:])
    nc.compile()
    return nc


def _build_b(no_x_dma=False, no_wt_dma=False, no_lora=False, no_out=False,
             wt_bufs=6, out_bufs=6, x_bufs=KT + 2, reps=1):
    """Orientation B: W slices stationary, resident x moving, out computed
    transposed ([DOUT, SEQ]) and transposed back on host."""
    nc = bacc.Bacc("TRN2", target_bir_lowering=False, debug=False)
    xT = nc.dram_tensor("xT", [DIN, SEQ], F32R, kind="ExternalInput")
    wT = nc.dram_tensor("wT", [DIN, DOUT], F32R, kind="ExternalInput")
    xaT = nc.dram_tensor("xaT", [RANK + 1, SEQ], F32R, kind="ExternalInput")
    bB = nc.dram_tensor("bB", [RANK + 1, DOUT], F32R, kind="ExternalInput")
    outT = nc.dram_tensor("outT", [DOUT, SEQ], F32, kind="ExternalOutput")

    with tile.TileContext(nc) as tc:
        with (
            tc.tile_pool(name="xblk", bufs=x_bufs) as xpool,
            tc.tile_pool(name="wt", bufs=wt_bufs) as wpool,
            tc.tile_pool(name="lora", bufs=1) as lpool,
            tc.tile_pool(name="outp", bufs=out_bufs) as opool,
            tc.tile_pool(name="psum", bufs=8, space="PSUM") as ppool,
        ):
            xa_sb = lpool.tile([RANK + 1, SEQ], F32R, tag="xa")
            nc.sync.dma_start(xa_sb[:], xaT[:, :])
            bB_sb = lpool.tile([RANK + 1, DOUT], F32R, tag="bB")
            nc.sync.dma_start(bB_sb[:], bB[:, :])

            def load_x(k, m0):
                xt = xpool.tile([128, M_BLK], F32R, name="x", tag="x")
                nc.sync.dma_start(
                    xt[:], xT[k * 128:(k + 1) * 128, m0:m0 + M_BLK])
                return xt

            import contextlib
            rep_ctx = tc.For_i(0, reps, 1) if reps > 1 else contextlib.nullcontext()
            with rep_ctx:
              for half in range(N_MBLK):
                m0 = half * M_BLK
                xtiles = [None] * KT
                for og in range(OT):
                    og0 = og * 512
                    psums = [ppool.tile([128, 512], F32, name="ps", tag="ps")
                             for _ in range(8)]
                    for k in range(KT):
                        if xtiles[k] is None:
                            xtiles[k] = load_x(k, m0)
                        wt = wpool.tile([128, 512], F32R, name="w", tag="w")
                        nc.sync.dma_start(
                            wt[:], wT[k * 128:(k + 1) * 128, og0:og0 + 512])
                        for oi in range(4):
                            for mc in range(2):
                                nc.tensor.matmul(
                                    psums[oi * 2 + mc][:],
                                    wt[:, oi * 128:(oi + 1) * 128],
                                    xtiles[k][:, mc * 512:(mc + 1) * 512],
                                    start=(k == 0), stop=False)
                    for oi in range(4):
                        for mc in range(2):
                            nc.tensor.matmul(
                                psums[oi * 2 + mc][:],
                                bB_sb[:, og0 + oi * 128:og0 + (oi + 1) * 128],
                                xa_sb[:, m0 + mc * 512:m0 + (mc + 1) * 512],
                                start=False, stop=True)
                            ot = opool.tile([128, 512], F32, name="o", tag="o")
                            nc.vector.tensor_copy(ot[:], psums[oi * 2 + mc][:])
                            nc.sync.dma_start(
                                outT[og0 + oi * 128:og0 + (oi + 1) * 128,
                                     m0 + mc * 512:m0 + (mc + 1) * 512],
                                ot[:])
    nc.compile()
    return nc


BF16 = mybir.dt.bfloat16
OG = 16                  # out-col groups of 256 (variant C)


def _build_c():
    """Variant C: single fused GEMM out.T = W'.T @ x.T in bf16.

    Host folds A@B into W (W' = W.T + A@B) and pre-tiles W' so each
    out-row group of 256 streams as one contiguous [128, 8192] DMA.
    All 32 x k-tiles ([128, 2048] bf16) stay resident in SBUF; W is
    read exactly once. Bias is added during the PSUM->SBUF drain.
    """
    nc = bacc.Bacc("TRN2", target_bir_lowering=False, debug=False)
    xT = nc.dram_tensor("xT", [DIN, SEQ], BF16, kind="ExternalInput")
    wR = nc.dram_tensor("wR", [OG * 128, KT * 256], BF16, kind="ExternalInput")
    biasR = nc.dram_tensor("biasR", [128, 2 * OG], F32, kind="ExternalInput")
    outT = nc.dram_tensor("outT", [DOUT, SEQ], F32, kind="ExternalOutput")

    with tile.TileContext(nc) as tc:
        with (
            tc.tile_pool(name="xblk", bufs=KT) as xpool,
            tc.tile_pool(name="wt", bufs=2) as wpool,
            tc.tile_pool(name="bias", bufs=1) as lpool,
            tc.tile_pool(name="outp", bufs=6) as opool,
            tc.tile_pool(name="psum", bufs=8, space="PSUM") as ppool,
        ):
            bias_sb = lpool.tile([128, 2 * OG], F32, tag="bias")
            nc.sync.dma_start(bias_sb[:], biasR[:, :])

            xtiles = [None] * KT
            for og in range(OG):
                wsb = wpool.tile([128, KT * 256], BF16, name="w", tag="w")
                nc.sync.dma_start(wsb[:], wR[og * 128:(og + 1) * 128, :])
                psums = [ppool.tile([128, 512], F32, name="ps", tag="ps")
                         for _ in range(8)]
                for k in range(KT):
                    if xtiles[k] is None:
                        xt = xpool.tile([128, SEQ], BF16, name="x", tag="x")
                        nc.sync.dma_start(
                            xt[:], xT[k * 128:(k + 1) * 128, :])
                        xtiles[k] = xt
                    for oi in range(2):
                        st = wsb[:, k * 256 + oi * 128:k * 256 + (oi + 1) * 128]
                        for mc in range(4):
                            nc.tensor.matmul(
                                psums[oi * 4 + mc][:],
                                st,
                                xtiles[k][:, mc * 512:(mc + 1) * 512],
                                start=(k == 0), stop=(k == KT - 1))
                for oi in range(2):
                    for mc in range(4):
                        ot = opool.tile([128, 512], F32, name="o", tag="o")
                        nc.vector.tensor_scalar_add(
                            ot[:], psums[oi * 4 + mc][:],
                            bias_sb[:, og * 2 + oi:og * 2 + oi + 1])
                        nc.sync.dma_start(
                            outT[og * 256 + oi * 128:og * 256 + (oi + 1) * 128,
                                 mc * 512:(mc + 1) * 512],
                            ot[:])
    nc.compile()
    return nc


VARIANT = "B"


def kernel(x, A, B, weight, bias):
    if not _nc_cache:
        _nc_cache.append(
            {"A": _build, "B": _build_b, "C": _build_c}[VARIANT]())
    nc = _nc_cache[0]

    x = np.asarray(x, dtype=np.float32)
    A = np.asarray(A, dtype=np.float32)
    B = np.asarray(B, dtype=np.float32)
    weight = np.asarray(weight, dtype=np.float32)
    bias = np.asarray(bias, dtype=np.float32)

    if VARIANT == "C":
        from ml_dtypes import bfloat16
        wTf = weight.T + A @ B                               # [DIN, DOUT]
        wR = np.ascontiguousarray(
            wTf.reshape(KT, 128, OG, 256).transpose(2, 1, 0, 3)
               .reshape(OG * 128, KT * 256)).astype(bfloat16)
        biasR = np.ascontiguousarray(
            bias.reshape(2 * OG, 128).T.astype(np.float32))
        in_maps = []
        for b in range(N_CORES):
            xTb = np.ascontiguousarray(x[b].T).astype(bfloat16)
            in_maps.append({"xT": xTb, "wR": wR, "biasR": biasR})
        res = run_bass_kernel_spmd(nc, in_maps, core_ids=list(range(N_CORES)))
        last_result.clear()
        last_result.append(res)
        return np.stack(
            [np.ascontiguousarray(r["outT"].T) for r in res.results], axis=0)

    wT = np.ascontiguousarray(weight.T)                      # [DIN, DOUT]
    bB = np.concatenate([B, bias[None, :]], axis=0)          # [RANK+1, DOUT]
    bB = np.ascontiguousarray(bB, dtype=np.float32)

    in_maps = []
    for b in range(N_CORES):
        xb = x[b]                                            # [SEQ, DIN]
        xTb = np.ascontiguousarray(xb.T)                     # [DIN, SEQ]
        xa = xb @ A                                          # [SEQ, RANK]
        xaT = np.concatenate(
            [np.ascontiguousarray(xa.T),
             np.ones((1, SEQ), dtype=np.float32)], axis=0)   # [RANK+1, SEQ]
        in_maps.append({"xT": xTb, "wT": wT, "xaT": xaT, "bB": bB})

    res = run_bass_kernel_spmd(nc, in_maps, core_ids=list(range(N_CORES)))
    last_result.clear()
    last_result.append(res)
    if VARIANT == "B":
        return np.stack(
            [np.ascontiguousarray(r["outT"].T) for r in res.results], axis=0)
    return np.stack([r["out"] for r in res.results], axis=0)


last_result = []



# revision 5
# speedup vs baseline: 1.1317x; 1.1317x over previous
"""LoRA layer (x @ W.T + (x@A)@B + bias) on 8 trn2 NeuronCores.

Data-parallel: core b computes batch b's (2048, 4096) output slice.
Per-core device work is a single fused matmul: the (4096-deep) base
projection accumulates 32 K=128 fp32r steps into PSUM, then one extra
K=17 step adds the low-rank correction and bias ((x@A | 1) @ (B ; bias)).
x@A (rank 16, 0.4% of FLOPs) and all transposes are host-side prep.
"""
import numpy as np

import concourse.mybir as mybir
import concourse.tile as tile
from concourse import bacc
from concourse.bass_utils import run_bass_kernel_spmd

BATCH, SEQ, DIN, DOUT, RANK = 8, 2048, 4096, 4096, 16
N_CORES = 8

KT = DIN // 128          # 32 contraction tiles
M_BLK = 1024             # tokens resident per block
N_MBLK = SEQ // M_BLK    # 2 blocks
MT_PER_BLK = M_BLK // 128  # 8 m-tiles -> 8 PSUM banks
OT = DOUT // 512         # 8 output-column tiles
F32R = mybir.dt.float32r
F32 = mybir.dt.float32

_nc_cache = []


def _build(no_x_dma=False, no_wt_dma=False, no_lora=False, no_out=False,
           wt_bufs=6, out_bufs=6, x_bufs=KT + 2, x_in_first_o=True, reps=1):
    nc = bacc.Bacc("TRN2", target_bir_lowering=False, debug=False)
    xT = nc.dram_tensor("xT", [DIN, SEQ], F32R, kind="ExternalInput")
    wT = nc.dram_tensor("wT", [DIN, DOUT], F32R, kind="ExternalInput")
    xaT = nc.dram_tensor("xaT", [RANK + 1, SEQ], F32R, kind="ExternalInput")
    bB = nc.dram_tensor("bB", [RANK + 1, DOUT], F32R, kind="ExternalInput")
    out = nc.dram_tensor("out", [SEQ, DOUT], F32, kind="ExternalOutput")

    with tile.TileContext(nc) as tc:
        with (
            tc.tile_pool(name="xblk", bufs=x_bufs) as xpool,
            tc.tile_pool(name="wt", bufs=wt_bufs) as wpool,
            tc.tile_pool(name="lora", bufs=1) as lpool,
            tc.tile_pool(name="outp", bufs=out_bufs) as opool,
            tc.tile_pool(name="psum", bufs=8, space="PSUM") as ppool,
        ):
            xa_sb = lpool.tile([RANK + 1, SEQ], F32R, tag="xa")
            nc.sync.dma_start(xa_sb[:], xaT[:, :])
            bB_sb = lpool.tile([RANK + 1, DOUT], F32R, tag="bB")
            nc.sync.dma_start(bB_sb[:], bB[:, :])

            def load_x(xpool, k, m0):
                xt = xpool.tile([128, M_BLK], F32R, name="x", tag="x")
                if no_x_dma:
                    nc.sync.dma_start(xt[:, :4], xT[k * 128:(k + 1) * 128, :4])
                else:
                    nc.sync.dma_start(
                        xt[:], xT[k * 128:(k + 1) * 128, m0:m0 + M_BLK])
                return xt

            import contextlib
            rep_ctx = tc.For_i(0, reps, 1) if reps > 1 else contextlib.nullcontext()
            with rep_ctx:
              for blk in range(N_MBLK):
                  m0 = blk * M_BLK
                  xtiles = [None] * KT
                  if not x_in_first_o:
                      for k in range(KT):
                          xtiles[k] = load_x(xpool, k, m0)
                  for o in range(OT):
                      o0 = o * 512
                      psums = [ppool.tile([128, 512], F32, name="ps", tag="ps")
                               for _ in range(MT_PER_BLK)]
                      for k in range(KT):
                          if xtiles[k] is None:
                              xtiles[k] = load_x(xpool, k, m0)
                          wt = wpool.tile([128, 512], F32R, name="w", tag="w")
                          if no_wt_dma:
                              nc.sync.dma_start(
                                  wt[:, :4], wT[k * 128:(k + 1) * 128, :4])
                          else:
                              nc.sync.dma_start(
                                  wt[:], wT[k * 128:(k + 1) * 128, o0:o0 + 512])
                          for mt in range(MT_PER_BLK):
                              nc.tensor.matmul(
                                  psums[mt][:],
                                  xtiles[k][:, mt * 128:(mt + 1) * 128],
                                  wt[:],
                                  start=(k == 0), stop=(no_lora and k == KT - 1))
                      for mt in range(MT_PER_BLK):
                          ms = m0 + mt * 128
                          if not no_lora:
                              nc.tensor.matmul(
                                  psums[mt][:],
                                  xa_sb[:, ms:ms + 128],
                                  bB_sb[:, o0:o0 + 512],
                                  start=False, stop=True)
                          ot = opool.tile([128, 512], F32, name="o", tag="o")
                          nc.vector.tensor_copy(ot[:], psums[mt][:])
                          if not no_out:
                              nc.sync.dma_start(
                                  out[ms:ms + 128, o0:o0 + 512], ot[:])
    nc.compile()
    return nc


def _build_b(no_x_dma=False, no_wt_dma=False, no_lora=False, no_out=False,
             wt_bufs=6, out_bufs=6, x_bufs=KT + 2, reps=1):
    """Orientation B: W slices stationary, resident x moving, out computed
    transposed ([DOUT, SEQ]) and transposed back on host."""
    nc = bacc.Bacc("TRN2", target_bir_lowering=False, debug=False)
    xT = nc.dram_tensor("xT", [DIN, SEQ], F32R, kind="ExternalInput")
    wT = nc.dram_tensor("wT", [DIN, DOUT], F32R, kind="ExternalInput")
    xaT = nc.dram_tensor("xaT", [RANK + 1, SEQ], F32R, kind="ExternalInput")
    bB = nc.dram_tensor("bB", [RANK + 1, DOUT], F32R, kind="ExternalInput")
    outT = nc.dram_tensor("outT", [DOUT, SEQ], F32, kind="ExternalOutput")

    with tile.TileContext(nc) as tc:
        with (
            tc.tile_pool(name="xblk", bufs=x_bufs) as xpool,
            tc.tile_pool(name="wt", bufs=wt_bufs) as wpool,
            tc.tile_pool(name="lora", bufs=1) as lpool,
            tc.tile_pool(name="outp", bufs=out_bufs) as opool,
            tc.tile_pool(name="psum", bufs=8, space="PSUM") as ppool,
        ):
            xa_sb = lpool.tile([RANK + 1, SEQ], F32R, tag="xa")
            nc.sync.dma_start(xa_sb[:], xaT[:, :])
            bB_sb = lpool.tile([RANK + 1, DOUT], F32R, tag="bB")
            nc.sync.dma_start(bB_sb[:], bB[:, :])

            def load_x(k, m0):
                xt = xpool.tile([128, M_BLK], F32R, name="x", tag="x")
                nc.sync.dma_start(
                    xt[:], xT[k * 128:(k + 1) * 128, m0:m0 + M_BLK])
                return xt

            import contextlib
            rep_ctx = tc.For_i(0, reps, 1) if reps > 1 else contextlib.nullcontext()
            with rep_ctx:
              for half in range(N_MBLK):
                m0 = half * M_BLK
                xtiles = [None] * KT
                for og in range(OT):
                    og0 = og * 512
                    psums = [ppool.tile([128, 512], F32, name="ps", tag="ps")
                             for _ in range(8)]
                    for k in range(KT):
                        if xtiles[k] is None:
                            xtiles[k] = load_x(k, m0)
                        wt = wpool.tile([128, 512], F32R, name="w", tag="w")
                        nc.sync.dma_start(
                            wt[:], wT[k * 128:(k + 1) * 128, og0:og0 + 512])
                        for oi in range(4):
                            for mc in range(2):
                                nc.tensor.matmul(
                                    psums[oi * 2 + mc][:],
                                    wt[:, oi * 128:(oi + 1) * 128],
                                    xtiles[k][:, mc * 512:(mc + 1) * 512],
                                    start=(k == 0), stop=False)
                    for oi in range(4):
                        for mc in range(2):
                            nc.tensor.matmul(
                                psums[oi * 2 + mc][:],
                                bB_sb[:, og0 + oi * 128:og0 + (oi + 1) * 128],
                                xa_sb[:, m0 + mc * 512:m0 + (mc + 1) * 512],
                                start=False, stop=True)
                            ot = opool.tile([128, 512], F32, name="o", tag="o")
                            nc.vector.tensor_copy(ot[:], psums[oi * 2 + mc][:])
                            nc.sync.dma_start(
                                outT[og0 + oi * 128:og0 + (oi + 1) * 128,
                                     m0 + mc * 512:m0 + (mc + 1) * 512],
                                ot[:])
    nc.compile()
    return nc


BF16 = mybir.dt.bfloat16
OG = 16                  # out-col groups of 256 (variant C)


def _build_c():
    """Variant C: single fused GEMM out.T = W'.T @ x.T in bf16.

    Host folds A@B into W (W' = W.T + A@B) and pre-tiles W' so each
    out-row group of 256 streams as one contiguous [128, 8192] DMA.
    All 32 x k-tiles ([128, 2048] bf16) stay resident in SBUF; W is
    read exactly once. Bias is added during the PSUM->SBUF drain.
    """
    nc = bacc.Bacc("TRN2", target_bir_lowering=False, debug=False)
    xT = nc.dram_tensor("xT", [DIN, SEQ], BF16, kind="ExternalInput")
    wR = nc.dram_tensor("wR", [OG * 128, KT * 256], BF16, kind="ExternalInput")
    biasR = nc.dram_tensor("biasR", [128, 2 * OG], F32, kind="ExternalInput")
    outT = nc.dram_tensor("outT", [DOUT, SEQ], F32, kind="ExternalOutput")

    with tile.TileContext(nc) as tc:
        with (
            tc.tile_pool(name="xblk", bufs=KT) as xpool,
            tc.tile_pool(name="wt", bufs=2) as wpool,
            tc.tile_pool(name="bias", bufs=1) as lpool,
            tc.tile_pool(name="outp", bufs=6) as opool,
            tc.tile_pool(name="psum", bufs=8, space="PSUM") as ppool,
        ):
            bias_sb = lpool.tile([128, 2 * OG], F32, tag="bias")
            nc.sync.dma_start(bias_sb[:], biasR[:, :])

            xtiles = [None] * KT
            for og in range(OG):
                wsb = wpool.tile([128, KT * 256], BF16, name="w", tag="w")
                nc.sync.dma_start(wsb[:], wR[og * 128:(og + 1) * 128, :])
                psums = [ppool.tile([128, 512], F32, name="ps", tag="ps")
                         for _ in range(8)]
                for k in range(KT):
                    if xtiles[k] is None:
                        xt = xpool.tile([128, SEQ], BF16, name="x", tag="x")
                        nc.sync.dma_start(
                            xt[:], xT[k * 128:(k + 1) * 128, :])
                        xtiles[k] = xt
                    for oi in range(2):
                        st = wsb[:, k * 256 + oi * 128:k * 256 + (oi + 1) * 128]
                        for mc in range(4):
                            nc.tensor.matmul(
                                psums[oi * 4 + mc][:],
                                st,
                                xtiles[k][:, mc * 512:(mc + 1) * 512],
                                start=(k == 0), stop=(k == KT - 1))
                for oi in range(2):
                    for mc in range(4):
                        ot = opool.tile([128, 512], F32, name="o", tag="o")
                        nc.vector.tensor_scalar_add(
                            ot[:], psums[oi * 4 + mc][:],
                            bias_sb[:, og * 2 + oi:og * 2 + oi + 1])
                        nc.sync.dma_start(
                            outT[og * 256 + oi * 128:og * 256 + (oi + 1) * 128,
                                 mc * 512:(mc + 1) * 512],
                            ot[:])
    nc.compile()
    return nc


VARIANT = "C"


def kernel(x, A, B, weight, bias):
    if not _nc_cache:
        _nc_cache.append(
            {"A": _build, "B": _build_b, "C": _build_c}[VARIANT]())
    nc = _nc_cache[0]

    x = np.asarray(x, dtype=np.float32)
    A = np.asarray(A, dtype=np.float32)
    B = np.asarray(B, dtype=np.float32)
    weight = np.asarray(weight, dtype=np.float32)
    bias = np.asarray(bias, dtype=np.float32)

    if VARIANT == "C":
        from ml_dtypes import bfloat16
        wTf = weight.T + A @ B                               # [DIN, DOUT]
        wR = np.ascontiguousarray(
            wTf.reshape(KT, 128, OG, 256).transpose(2, 1, 0, 3)
               .reshape(OG * 128, KT * 256)).astype(bfloat16)
        biasR = np.ascontiguousarray(
            bias.reshape(2 * OG, 128).T.astype(np.float32))
        in_maps = []
        for b in range(N_CORES):
            xTb = np.ascontiguousarray(x[b].T).astype(bfloat16)
            in_maps.append({"xT": xTb, "wR": wR, "biasR": biasR})
        res = run_bass_kernel_spmd(nc, in_maps, core_ids=list(range(N_CORES)))
        last_result.clear()
        last_result.append(res)
        return np.stack(
            [np.ascontiguousarray(r["outT"].T) for r in res.results], axis=0)

    wT = np.ascontiguousarray(weight.T)                      # [DIN, DOUT]
    bB = np.concatenate([B, bias[None, :]], axis=0)          # [RANK+1, DOUT]
    bB = np.ascontiguousarray(bB, dtype=np.float32)

    in_maps = []
    for b in range(N_CORES):
        xb = x[b]                                            # [SEQ, DIN]
        xTb = np.ascontiguousarray(xb.T)                     # [DIN, SEQ]
        xa = xb @ A                                          # [SEQ, RANK]
        xaT = np.concatenate(
            [np.ascontiguousarray(xa.T),
             np.ones((1, SEQ), dtype=np.float32)], axis=0)   # [RANK+1, SEQ]
        in_maps.append({"xT": xTb, "wT": wT, "xaT": xaT, "bB": bB})

    res = run_bass_kernel_spmd(nc, in_maps, core_ids=list(range(N_CORES)))
    last_result.clear()
    last_result.append(res)
    if VARIANT == "B":
        return np.stack(
            [np.ascontiguousarray(r["outT"].T) for r in res.results], axis=0)
    return np.stack([r["out"] for r in res.results], axis=0)


last_result = []



# revision 7
# speedup vs baseline: 1.1318x; 1.0000x over previous
"""LoRA layer (x @ W.T + (x@A)@B + bias) on 8 trn2 NeuronCores.

Data-parallel: core b computes batch b's (2048, 4096) output slice.
Per-core device work is a single fused matmul: the (4096-deep) base
projection accumulates 32 K=128 fp32r steps into PSUM, then one extra
K=17 step adds the low-rank correction and bias ((x@A | 1) @ (B ; bias)).
x@A (rank 16, 0.4% of FLOPs) and all transposes are host-side prep.
"""
import numpy as np

import concourse.mybir as mybir
import concourse.tile as tile
from concourse import bacc
from concourse.bass_utils import run_bass_kernel_spmd

BATCH, SEQ, DIN, DOUT, RANK = 8, 2048, 4096, 4096, 16
N_CORES = 8

KT = DIN // 128          # 32 contraction tiles
M_BLK = 1024             # tokens resident per block
N_MBLK = SEQ // M_BLK    # 2 blocks
MT_PER_BLK = M_BLK // 128  # 8 m-tiles -> 8 PSUM banks
OT = DOUT // 512         # 8 output-column tiles
F32R = mybir.dt.float32r
F32 = mybir.dt.float32

_nc_cache = []


def _build(no_x_dma=False, no_wt_dma=False, no_lora=False, no_out=False,
           wt_bufs=6, out_bufs=6, x_bufs=KT + 2, x_in_first_o=True, reps=1):
    nc = bacc.Bacc("TRN2", target_bir_lowering=False, debug=False)
    xT = nc.dram_tensor("xT", [DIN, SEQ], F32R, kind="ExternalInput")
    wT = nc.dram_tensor("wT", [DIN, DOUT], F32R, kind="ExternalInput")
    xaT = nc.dram_tensor("xaT", [RANK + 1, SEQ], F32R, kind="ExternalInput")
    bB = nc.dram_tensor("bB", [RANK + 1, DOUT], F32R, kind="ExternalInput")
    out = nc.dram_tensor("out", [SEQ, DOUT], F32, kind="ExternalOutput")

    with tile.TileContext(nc) as tc:
        with (
            tc.tile_pool(name="xblk", bufs=x_bufs) as xpool,
            tc.tile_pool(name="wt", bufs=wt_bufs) as wpool,
            tc.tile_pool(name="lora", bufs=1) as lpool,
            tc.tile_pool(name="outp", bufs=out_bufs) as opool,
            tc.tile_pool(name="psum", bufs=8, space="PSUM") as ppool,
        ):
            xa_sb = lpool.tile([RANK + 1, SEQ], F32R, tag="xa")
            nc.sync.dma_start(xa_sb[:], xaT[:, :])
            bB_sb = lpool.tile([RANK + 1, DOUT], F32R, tag="bB")
            nc.sync.dma_start(bB_sb[:], bB[:, :])

            def load_x(xpool, k, m0):
                xt = xpool.tile([128, M_BLK], F32R, name="x", tag="x")
                if no_x_dma:
                    nc.sync.dma_start(xt[:, :4], xT[k * 128:(k + 1) * 128, :4])
                else:
                    nc.sync.dma_start(
                        xt[:], xT[k * 128:(k + 1) * 128, m0:m0 + M_BLK])
                return xt

            import contextlib
            rep_ctx = tc.For_i(0, reps, 1) if reps > 1 else contextlib.nullcontext()
            with rep_ctx:
              for blk in range(N_MBLK):
                  m0 = blk * M_BLK
                  xtiles = [None] * KT
                  if not x_in_first_o:
                      for k in range(KT):
                          xtiles[k] = load_x(xpool, k, m0)
                  for o in range(OT):
                      o0 = o * 512
                      psums = [ppool.tile([128, 512], F32, name="ps", tag="ps")
                               for _ in range(MT_PER_BLK)]
                      for k in range(KT):
                          if xtiles[k] is None:
                              xtiles[k] = load_x(xpool, k, m0)
                          wt = wpool.tile([128, 512], F32R, name="w", tag="w")
                          if no_wt_dma:
                              nc.sync.dma_start(
                                  wt[:, :4], wT[k * 128:(k + 1) * 128, :4])
                          else:
                              nc.sync.dma_start(
                                  wt[:], wT[k * 128:(k + 1) * 128, o0:o0 + 512])
                          for mt in range(MT_PER_BLK):
                              nc.tensor.matmul(
                                  psums[mt][:],
                                  xtiles[k][:, mt * 128:(mt + 1) * 128],
                                  wt[:],
                                  start=(k == 0), stop=(no_lora and k == KT - 1))
                      for mt in range(MT_PER_BLK):
                          ms = m0 + mt * 128
                          if not no_lora:
                              nc.tensor.matmul(
                                  psums[mt][:],
                                  xa_sb[:, ms:ms + 128],
                                  bB_sb[:, o0:o0 + 512],
                                  start=False, stop=True)
                          ot = opool.tile([128, 512], F32, name="o", tag="o")
                          nc.vector.tensor_copy(ot[:], psums[mt][:])
                          if not no_out:
                              nc.sync.dma_start(
                                  out[ms:ms + 128, o0:o0 + 512], ot[:])
    nc.compile()
    return nc


def _build_b(no_x_dma=False, no_wt_dma=False, no_lora=False, no_out=False,
             wt_bufs=6, out_bufs=6, x_bufs=KT + 2, reps=1):
    """Orientation B: W slices stationary, resident x moving, out computed
    transposed ([DOUT, SEQ]) and transposed back on host."""
    nc = bacc.Bacc("TRN2", target_bir_lowering=False, debug=False)
    xT = nc.dram_tensor("xT", [DIN, SEQ], F32R, kind="ExternalInput")
    wT = nc.dram_tensor("wT", [DIN, DOUT], F32R, kind="ExternalInput")
    xaT = nc.dram_tensor("xaT", [RANK + 1, SEQ], F32R, kind="ExternalInput")
    bB = nc.dram_tensor("bB", [RANK + 1, DOUT], F32R, kind="ExternalInput")
    outT = nc.dram_tensor("outT", [DOUT, SEQ], F32, kind="ExternalOutput")

    with tile.TileContext(nc) as tc:
        with (
            tc.tile_pool(name="xblk", bufs=x_bufs) as xpool,
            tc.tile_pool(name="wt", bufs=wt_bufs) as wpool,
            tc.tile_pool(name="lora", bufs=1) as lpool,
            tc.tile_pool(name="outp", bufs=out_bufs) as opool,
            tc.tile_pool(name="psum", bufs=8, space="PSUM") as ppool,
        ):
            xa_sb = lpool.tile([RANK + 1, SEQ], F32R, tag="xa")
            nc.sync.dma_start(xa_sb[:], xaT[:, :])
            bB_sb = lpool.tile([RANK + 1, DOUT], F32R, tag="bB")
            nc.sync.dma_start(bB_sb[:], bB[:, :])

            def load_x(k, m0):
                xt = xpool.tile([128, M_BLK], F32R, name="x", tag="x")
                nc.sync.dma_start(
                    xt[:], xT[k * 128:(k + 1) * 128, m0:m0 + M_BLK])
                return xt

            import contextlib
            rep_ctx = tc.For_i(0, reps, 1) if reps > 1 else contextlib.nullcontext()
            with rep_ctx:
              for half in range(N_MBLK):
                m0 = half * M_BLK
                xtiles = [None] * KT
                for og in range(OT):
                    og0 = og * 512
                    psums = [ppool.tile([128, 512], F32, name="ps", tag="ps")
                             for _ in range(8)]
                    for k in range(KT):
                        if xtiles[k] is None:
                            xtiles[k] = load_x(k, m0)
                        wt = wpool.tile([128, 512], F32R, name="w", tag="w")
                        nc.sync.dma_start(
                            wt[:], wT[k * 128:(k + 1) * 128, og0:og0 + 512])
                        for oi in range(4):
                            for mc in range(2):
                                nc.tensor.matmul(
                                    psums[oi * 2 + mc][:],
                                    wt[:, oi * 128:(oi + 1) * 128],
                                    xtiles[k][:, mc * 512:(mc + 1) * 512],
                                    start=(k == 0), stop=False)
                    for oi in range(4):
                        for mc in range(2):
                            nc.tensor.matmul(
                                psums[oi * 2 + mc][:],
                                bB_sb[:, og0 + oi * 128:og0 + (oi + 1) * 128],
                                xa_sb[:, m0 + mc * 512:m0 + (mc + 1) * 512],
                                start=False, stop=True)
                            ot = opool.tile([128, 512], F32, name="o", tag="o")
                            nc.vector.tensor_copy(ot[:], psums[oi * 2 + mc][:])
                            nc.sync.dma_start(
                                outT[og0 + oi * 128:og0 + (oi + 1) * 128,
                                     m0 + mc * 512:m0 + (mc + 1) * 512],
                                ot[:])
    nc.compile()
    return nc


BF16 = mybir.dt.bfloat16
OG = 16                  # out-col groups of 256 (variant C)


def _build_c():
    """Variant C: single fused GEMM out.T = W'.T @ x.T in bf16.

    Host folds A@B into W (W' = W.T + A@B) and pre-tiles W' so each
    out-row group of 256 streams as one contiguous [128, 8192] DMA.
    All 32 x k-tiles ([128, 2048] bf16) stay resident in SBUF; W is
    read exactly once. Bias is added during the PSUM->SBUF drain.
    """
    nc = bacc.Bacc("TRN2", target_bir_lowering=False, debug=False)
    xT = nc.dram_tensor("xT", [DIN, SEQ], BF16, kind="ExternalInput")
    wR = nc.dram_tensor("wR", [OG * 128, KT * 256], BF16, kind="ExternalInput")
    biasR = nc.dram_tensor("biasR", [128, 2 * OG], F32, kind="ExternalInput")
    outT = nc.dram_tensor("outT", [DOUT, SEQ], F32, kind="ExternalOutput")

    with tile.TileContext(nc) as tc:
        with (
            tc.tile_pool(name="xblk", bufs=KT) as xpool,
            tc.tile_pool(name="wt", bufs=8) as wpool,
            tc.tile_pool(name="bias", bufs=1) as lpool,
            tc.tile_pool(name="outp", bufs=6) as opool,
            tc.tile_pool(name="psum", bufs=8, space="PSUM") as ppool,
        ):
            bias_sb = lpool.tile([128, 2 * OG], F32, tag="bias")
            nc.sync.dma_start(bias_sb[:], biasR[:, :])

            KC = KT // 4                 # k-steps per W chunk
            CW = KC * 256                # chunk width in wR cols
            xtiles = [None] * KT
            for og in range(OG):
                wchunks = []
                for c in range(4):
                    wc = wpool.tile([128, CW], BF16, name="w", tag="w")
                    nc.sync.dma_start(
                        wc[:], wR[og * 128:(og + 1) * 128,
                                  c * CW:(c + 1) * CW])
                    wchunks.append(wc)
                psums = [ppool.tile([128, 512], F32, name="ps", tag="ps")
                         for _ in range(8)]
                for k in range(KT):
                    if xtiles[k] is None:
                        xt = xpool.tile([128, SEQ], BF16, name="x", tag="x")
                        nc.sync.dma_start(
                            xt[:], xT[k * 128:(k + 1) * 128, :])
                        xtiles[k] = xt
                    wsb = wchunks[k // KC]
                    kc = k % KC
                    for oi in range(2):
                        st = wsb[:, kc * 256 + oi * 128:kc * 256 + (oi + 1) * 128]
                        for mc in range(4):
                            nc.tensor.matmul(
                                psums[oi * 4 + mc][:],
                                st,
                                xtiles[k][:, mc * 512:(mc + 1) * 512],
                                start=(k == 0), stop=(k == KT - 1))
                for oi in range(2):
                    for mc in range(4):
                        ot = opool.tile([128, 512], F32, name="o", tag="o")
                        nc.vector.tensor_scalar_add(
                            ot[:], psums[oi * 4 + mc][:],
                            bias_sb[:, og * 2 + oi:og * 2 + oi + 1])
                        nc.sync.dma_start(
                            outT[og * 256 + oi * 128:og * 256 + (oi + 1) * 128,
                                 mc * 512:(mc + 1) * 512],
                            ot[:])
    nc.compile()
    return nc


VARIANT = "C"


def kernel(x, A, B, weight, bias):
    if not _nc_cache:
        _nc_cache.append(
            {"A": _build, "B": _build_b, "C": _build_c}[VARIANT]())
    nc = _nc_cache[0]

    x = np.asarray(x, dtype=np.float32)
    A = np.asarray(A, dtype=np.float32)
    B = np.asarray(B, dtype=np.float32)
    weight = np.asarray(weight, dtype=np.float32)
    bias = np.asarray(bias, dtype=np.float32)

    if VARIANT == "C":
        from ml_dtypes import bfloat16
        wTf = weight.T + A @ B                               # [DIN, DOUT]
        wR = np.ascontiguousarray(
            wTf.reshape(KT, 128, OG, 256).transpose(2, 1, 0, 3)
               .reshape(OG * 128, KT * 256)).astype(bfloat16)
        biasR = np.ascontiguousarray(
            bias.reshape(2 * OG, 128).T.astype(np.float32))
        in_maps = []
        for b in range(N_CORES):
            xTb = np.ascontiguousarray(x[b].T).astype(bfloat16)
            in_maps.append({"xT": xTb, "wR": wR, "biasR": biasR})
        res = run_bass_kernel_spmd(nc, in_maps, core_ids=list(range(N_CORES)))
        last_result.clear()
        last_result.append(res)
        return np.stack(
            [np.ascontiguousarray(r["outT"].T) for r in res.results], axis=0)

    wT = np.ascontiguousarray(weight.T)                      # [DIN, DOUT]
    bB = np.concatenate([B, bias[None, :]], axis=0)          # [RANK+1, DOUT]
    bB = np.ascontiguousarray(bB, dtype=np.float32)

    in_maps = []
    for b in range(N_CORES):
        xb = x[b]                                            # [SEQ, DIN]
        xTb = np.ascontiguousarray(xb.T)                     # [DIN, SEQ]
        xa = xb @ A                                          # [SEQ, RANK]
        xaT = np.concatenate(
            [np.ascontiguousarray(xa.T),
             np.ones((1, SEQ), dtype=np.float32)], axis=0)   # [RANK+1, SEQ]
        in_maps.append({"xT": xTb, "wT": wT, "xaT": xaT, "bB": bB})

    res = run_bass_kernel_spmd(nc, in_maps, core_ids=list(range(N_CORES)))
    last_result.clear()
    last_result.append(res)
    if VARIANT == "B":
        return np.stack(
            [np.ascontiguousarray(r["outT"].T) for r in res.results], axis=0)
    return np.stack([r["out"] for r in res.results], axis=0)


last_result = []



# revision 8
# speedup vs baseline: 1.1435x; 1.0104x over previous
"""LoRA layer (x @ W.T + (x@A)@B + bias) on 8 trn2 NeuronCores.

Data-parallel: core b computes batch b's (2048, 4096) output slice.
Per-core device work is a single fused matmul: the (4096-deep) base
projection accumulates 32 K=128 fp32r steps into PSUM, then one extra
K=17 step adds the low-rank correction and bias ((x@A | 1) @ (B ; bias)).
x@A (rank 16, 0.4% of FLOPs) and all transposes are host-side prep.
"""
import numpy as np

import concourse.mybir as mybir
import concourse.tile as tile
from concourse import bacc
from concourse.bass_utils import run_bass_kernel_spmd

BATCH, SEQ, DIN, DOUT, RANK = 8, 2048, 4096, 4096, 16
N_CORES = 8

KT = DIN // 128          # 32 contraction tiles
M_BLK = 1024             # tokens resident per block
N_MBLK = SEQ // M_BLK    # 2 blocks
MT_PER_BLK = M_BLK // 128  # 8 m-tiles -> 8 PSUM banks
OT = DOUT // 512         # 8 output-column tiles
F32R = mybir.dt.float32r
F32 = mybir.dt.float32

_nc_cache = []


def _build(no_x_dma=False, no_wt_dma=False, no_lora=False, no_out=False,
           wt_bufs=6, out_bufs=6, x_bufs=KT + 2, x_in_first_o=True, reps=1):
    nc = bacc.Bacc("TRN2", target_bir_lowering=False, debug=False)
    xT = nc.dram_tensor("xT", [DIN, SEQ], F32R, kind="ExternalInput")
    wT = nc.dram_tensor("wT", [DIN, DOUT], F32R, kind="ExternalInput")
    xaT = nc.dram_tensor("xaT", [RANK + 1, SEQ], F32R, kind="ExternalInput")
    bB = nc.dram_tensor("bB", [RANK + 1, DOUT], F32R, kind="ExternalInput")
    out = nc.dram_tensor("out", [SEQ, DOUT], F32, kind="ExternalOutput")

    with tile.TileContext(nc) as tc:
        with (
            tc.tile_pool(name="xblk", bufs=x_bufs) as xpool,
            tc.tile_pool(name="wt", bufs=wt_bufs) as wpool,
            tc.tile_pool(name="lora", bufs=1) as lpool,
            tc.tile_pool(name="outp", bufs=out_bufs) as opool,
            tc.tile_pool(name="psum", bufs=8, space="PSUM") as ppool,
        ):
            xa_sb = lpool.tile([RANK + 1, SEQ], F32R, tag="xa")
            nc.sync.dma_start(xa_sb[:], xaT[:, :])
            bB_sb = lpool.tile([RANK + 1, DOUT], F32R, tag="bB")
            nc.sync.dma_start(bB_sb[:], bB[:, :])

            def load_x(xpool, k, m0):
                xt = xpool.tile([128, M_BLK], F32R, name="x", tag="x")
                if no_x_dma:
                    nc.sync.dma_start(xt[:, :4], xT[k * 128:(k + 1) * 128, :4])
                else:
                    nc.sync.dma_start(
                        xt[:], xT[k * 128:(k + 1) * 128, m0:m0 + M_BLK])
                return xt

            import contextlib
            rep_ctx = tc.For_i(0, reps, 1) if reps > 1 else contextlib.nullcontext()
            with rep_ctx:
              for blk in range(N_MBLK):
                  m0 = blk * M_BLK
                  xtiles = [None] * KT
                  if not x_in_first_o:
                      for k in range(KT):
                          xtiles[k] = load_x(xpool, k, m0)
                  for o in range(OT):
                      o0 = o * 512
                      psums = [ppool.tile([128, 512], F32, name="ps", tag="ps")
                               for _ in range(MT_PER_BLK)]
                      for k in range(KT):
                          if xtiles[k] is None:
                              xtiles[k] = load_x(xpool, k, m0)
                          wt = wpool.tile([128, 512], F32R, name="w", tag="w")
                          if no_wt_dma:
                              nc.sync.dma_start(
                                  wt[:, :4], wT[k * 128:(k + 1) * 128, :4])
                          else:
                              nc.sync.dma_start(
                                  wt[:], wT[k * 128:(k + 1) * 128, o0:o0 + 512])
                          for mt in range(MT_PER_BLK):
                              nc.tensor.matmul(
                                  psums[mt][:],
                                  xtiles[k][:, mt * 128:(mt + 1) * 128],
                                  wt[:],
                                  start=(k == 0), stop=(no_lora and k == KT - 1))
                      for mt in range(MT_PER_BLK):
                          ms = m0 + mt * 128
                          if not no_lora:
                              nc.tensor.matmul(
                                  psums[mt][:],
                                  xa_sb[:, ms:ms + 128],
                                  bB_sb[:, o0:o0 + 512],
                                  start=False, stop=True)
                          ot = opool.tile([128, 512], F32, name="o", tag="o")
                          nc.vector.tensor_copy(ot[:], psums[mt][:])
                          if not no_out:
                              nc.sync.dma_start(
                                  out[ms:ms + 128, o0:o0 + 512], ot[:])
    nc.compile()
    return nc


def _build_b(no_x_dma=False, no_wt_dma=False, no_lora=False, no_out=False,
             wt_bufs=6, out_bufs=6, x_bufs=KT + 2, reps=1):
    """Orientation B: W slices stationary, resident x moving, out computed
    transposed ([DOUT, SEQ]) and transposed back on host."""
    nc = bacc.Bacc("TRN2", target_bir_lowering=False, debug=False)
    xT = nc.dram_tensor("xT", [DIN, SEQ], F32R, kind="ExternalInput")
    wT = nc.dram_tensor("wT", [DIN, DOUT], F32R, kind="ExternalInput")
    xaT = nc.dram_tensor("xaT", [RANK + 1, SEQ], F32R, kind="ExternalInput")
    bB = nc.dram_tensor("bB", [RANK + 1, DOUT], F32R, kind="ExternalInput")
    outT = nc.dram_tensor("outT", [DOUT, SEQ], F32, kind="ExternalOutput")

    with tile.TileContext(nc) as tc:
        with (
            tc.tile_pool(name="xblk", bufs=x_bufs) as xpool,
            tc.tile_pool(name="wt", bufs=wt_bufs) as wpool,
            tc.tile_pool(name="lora", bufs=1) as lpool,
            tc.tile_pool(name="outp", bufs=out_bufs) as opool,
            tc.tile_pool(name="psum", bufs=8, space="PSUM") as ppool,
        ):
            xa_sb = lpool.tile([RANK + 1, SEQ], F32R, tag="xa")
            nc.sync.dma_start(xa_sb[:], xaT[:, :])
            bB_sb = lpool.tile([RANK + 1, DOUT], F32R, tag="bB")
            nc.sync.dma_start(bB_sb[:], bB[:, :])

            def load_x(k, m0):
                xt = xpool.tile([128, M_BLK], F32R, name="x", tag="x")
                nc.sync.dma_start(
                    xt[:], xT[k * 128:(k + 1) * 128, m0:m0 + M_BLK])
                return xt

            import contextlib
            rep_ctx = tc.For_i(0, reps, 1) if reps > 1 else contextlib.nullcontext()
            with rep_ctx:
              for half in range(N_MBLK):
                m0 = half * M_BLK
                xtiles = [None] * KT
                for og in range(OT):
                    og0 = og * 512
                    psums = [ppool.tile([128, 512], F32, name="ps", tag="ps")
                             for _ in range(8)]
                    for k in range(KT):
                        if xtiles[k] is None:
                            xtiles[k] = load_x(k, m0)
                        wt = wpool.tile([128, 512], F32R, name="w", tag="w")
                        nc.sync.dma_start(
                            wt[:], wT[k * 128:(k + 1) * 128, og0:og0 + 512])
                        for oi in range(4):
                            for mc in range(2):
                                nc.tensor.matmul(
                                    psums[oi * 2 + mc][:],
                                    wt[:, oi * 128:(oi + 1) * 128],
                                    xtiles[k][:, mc * 512:(mc + 1) * 512],
                                    start=(k == 0), stop=False)
                    for oi in range(4):
                        for mc in range(2):
                            nc.tensor.matmul(
                                psums[oi * 2 + mc][:],
                                bB_sb[:, og0 + oi * 128:og0 + (oi + 1) * 128],
                                xa_sb[:, m0 + mc * 512:m0 + (mc + 1) * 512],
                                start=False, stop=True)
                            ot = opool.tile([128, 512], F32, name="o", tag="o")
                            nc.vector.tensor_copy(ot[:], psums[oi * 2 + mc][:])
                            nc.sync.dma_start(
                                outT[og0 + oi * 128:og0 + (oi + 1) * 128,
                                     m0 + mc * 512:m0 + (mc + 1) * 512],
                                ot[:])
    nc.compile()
    return nc


BF16 = mybir.dt.bfloat16
OG = 16                  # out-col groups of 256 (variant C)


def _build_c():
    """Variant C: single fused GEMM out.T = W'.T @ x.T in bf16.

    Host folds A@B into W (W' = W.T + A@B) and pre-tiles W' so each
    out-row group of 256 streams as one contiguous [128, 8192] DMA.
    All 32 x k-tiles ([128, 2048] bf16) stay resident in SBUF; W is
    read exactly once. Bias is added during the PSUM->SBUF drain.
    """
    nc = bacc.Bacc("TRN2", target_bir_lowering=False, debug=False)
    xT = nc.dram_tensor("xT", [DIN, SEQ], BF16, kind="ExternalInput")
    wR = nc.dram_tensor("wR", [OG * 128, KT * 256], BF16, kind="ExternalInput")
    biasR = nc.dram_tensor("biasR", [128, 2 * OG], F32, kind="ExternalInput")
    outT = nc.dram_tensor("outT", [DOUT, SEQ], F32, kind="ExternalOutput")

    with tile.TileContext(nc) as tc:
        with (
            tc.tile_pool(name="xblk", bufs=KT) as xpool,
            tc.tile_pool(name="wt", bufs=8) as wpool,
            tc.tile_pool(name="bias", bufs=1) as lpool,
            tc.tile_pool(name="outp", bufs=6) as opool,
            tc.tile_pool(name="psum", bufs=8, space="PSUM") as ppool,
        ):
            bias_sb = lpool.tile([128, 2 * OG], F32, tag="bias")
            nc.sync.dma_start(bias_sb[:], biasR[:, :])

            KC = KT // 4                 # k-steps per W chunk
            CW = KC * 256                # chunk width in wR cols
            xtiles = [None] * KT

            def load_chunk(og, c):
                wc = wpool.tile([128, CW], BF16, name="w", tag="w")
                nc.sync.dma_start(
                    wc[:], wR[og * 128:(og + 1) * 128, c * CW:(c + 1) * CW])
                return wc

            # 32 groups of (og, oi); each accumulates in 4 PSUM banks so
            # group g+1's matmuls overlap group g's drains (other banks).
            for og in range(OG):
                first = og == 0
                wchunks = [load_chunk(og, 0)] + [None] * 3
                if not first:
                    for c in range(1, 4):
                        wchunks[c] = load_chunk(og, c)
                for oi in range(2):
                    psums = [ppool.tile([128, 512], F32, name="ps", tag="ps")
                             for _ in range(4)]
                    for k in range(KT):
                        if xtiles[k] is None:
                            xt = xpool.tile([128, SEQ], BF16, name="x", tag="x")
                            nc.sync.dma_start(
                                xt[:], xT[k * 128:(k + 1) * 128, :])
                            xtiles[k] = xt
                        c = k // KC
                        # first og: issue chunk c's DMA 2 k-steps ahead so
                        # the x-tile stream stays ahead of the PE
                        if first and oi == 0 and c < 3 and k == (c + 1) * KC - 2:
                            wchunks[c + 1] = load_chunk(og, c + 1)
                        wsb = wchunks[c]
                        kc = k % KC
                        st = wsb[:, kc * 256 + oi * 128:kc * 256 + (oi + 1) * 128]
                        for mc in range(4):
                            nc.tensor.matmul(
                                psums[mc][:],
                                st,
                                xtiles[k][:, mc * 512:(mc + 1) * 512],
                                start=(k == 0), stop=(k == KT - 1))
                    for mc in range(4):
                        ot = opool.tile([128, 512], F32, name="o", tag="o")
                        nc.vector.tensor_scalar_add(
                            ot[:], psums[mc][:],
                            bias_sb[:, og * 2 + oi:og * 2 + oi + 1])
                        nc.sync.dma_start(
                            outT[og * 256 + oi * 128:og * 256 + (oi + 1) * 128,
                                 mc * 512:(mc + 1) * 512],
                            ot[:])
    nc.compile()
    return nc


VARIANT = "C"


def kernel(x, A, B, weight, bias):
    if not _nc_cache:
        _nc_cache.append(
            {"A": _build, "B": _build_b, "C": _build_c}[VARIANT]())
    nc = _nc_cache[0]

    x = np.asarray(x, dtype=np.float32)
    A = np.asarray(A, dtype=np.float32)
    B = np.asarray(B, dtype=np.float32)
    weight = np.asarray(weight, dtype=np.float32)
    bias = np.asarray(bias, dtype=np.float32)

    if VARIANT == "C":
        from ml_dtypes import bfloat16
        wTf = weight.T + A @ B                               # [DIN, DOUT]
        wR = np.ascontiguousarray(
            wTf.reshape(KT, 128, OG, 256).transpose(2, 1, 0, 3)
               .reshape(OG * 128, KT * 256)).astype(bfloat16)
        biasR = np.ascontiguousarray(
            bias.reshape(2 * OG, 128).T.astype(np.float32))
        in_maps = []
        for b in range(N_CORES):
            xTb = np.ascontiguousarray(x[b].T).astype(bfloat16)
            in_maps.append({"xT": xTb, "wR": wR, "biasR": biasR})
        res = run_bass_kernel_spmd(nc, in_maps, core_ids=list(range(N_CORES)))
        last_result.clear()
        last_result.append(res)
        return np.stack(
            [np.ascontiguousarray(r["outT"].T) for r in res.results], axis=0)

    wT = np.ascontiguousarray(weight.T)                      # [DIN, DOUT]
    bB = np.concatenate([B, bias[None, :]], axis=0)          # [RANK+1, DOUT]
    bB = np.ascontiguousarray(bB, dtype=np.float32)

    in_maps = []
    for b in range(N_CORES):
        xb = x[b]                                            # [SEQ, DIN]
        xTb = np.ascontiguousarray(xb.T)                     # [DIN, SEQ]
        xa = xb @ A                                          # [SEQ, RANK]
        xaT = np.concatenate(
            [np.ascontiguousarray(xa.T),
             np.ones((1, SEQ), dtype=np.float32)], axis=0)   # [RANK+1, SEQ]
        in_maps.append({"xT": xTb, "wT": wT, "xaT": xaT, "bB": bB})

    res = run_bass_kernel_spmd(nc, in_maps, core_ids=list(range(N_CORES)))
    last_result.clear()
    last_result.append(res)
    if VARIANT == "B":
        return np.stack(
            [np.ascontiguousarray(r["outT"].T) for r in res.results], axis=0)
    return np.stack([r["out"] for r in res.results], axis=0)


last_result = []



# revision 10
# speedup vs baseline: 1.1454x; 1.0016x over previous
"""LoRA layer (x @ W.T + (x@A)@B + bias) on 8 trn2 NeuronCores.

Data-parallel: core b computes batch b's (2048, 4096) output slice.
Per-core device work is a single fused matmul: the (4096-deep) base
projection accumulates 32 K=128 fp32r steps into PSUM, then one extra
K=17 step adds the low-rank correction and bias ((x@A | 1) @ (B ; bias)).
x@A (rank 16, 0.4% of FLOPs) and all transposes are host-side prep.
"""
import numpy as np

import concourse.mybir as mybir
import concourse.tile as tile
from concourse import bacc
from concourse.bass_utils import run_bass_kernel_spmd

BATCH, SEQ, DIN, DOUT, RANK = 8, 2048, 4096, 4096, 16
N_CORES = 8

KT = DIN // 128          # 32 contraction tiles
M_BLK = 1024             # tokens resident per block
N_MBLK = SEQ // M_BLK    # 2 blocks
MT_PER_BLK = M_BLK // 128  # 8 m-tiles -> 8 PSUM banks
OT = DOUT // 512         # 8 output-column tiles
F32R = mybir.dt.float32r
F32 = mybir.dt.float32

_nc_cache = []


def _build(no_x_dma=False, no_wt_dma=False, no_lora=False, no_out=False,
           wt_bufs=6, out_bufs=6, x_bufs=KT + 2, x_in_first_o=True, reps=1):
    nc = bacc.Bacc("TRN2", target_bir_lowering=False, debug=False)
    xT = nc.dram_tensor("xT", [DIN, SEQ], F32R, kind="ExternalInput")
    wT = nc.dram_tensor("wT", [DIN, DOUT], F32R, kind="ExternalInput")
    xaT = nc.dram_tensor("xaT", [RANK + 1, SEQ], F32R, kind="ExternalInput")
    bB = nc.dram_tensor("bB", [RANK + 1, DOUT], F32R, kind="ExternalInput")
    out = nc.dram_tensor("out", [SEQ, DOUT], F32, kind="ExternalOutput")

    with tile.TileContext(nc) as tc:
        with (
            tc.tile_pool(name="xblk", bufs=x_bufs) as xpool,
            tc.tile_pool(name="wt", bufs=wt_bufs) as wpool,
            tc.tile_pool(name="lora", bufs=1) as lpool,
            tc.tile_pool(name="outp", bufs=out_bufs) as opool,
            tc.tile_pool(name="psum", bufs=8, space="PSUM") as ppool,
        ):
            xa_sb = lpool.tile([RANK + 1, SEQ], F32R, tag="xa")
            nc.sync.dma_start(xa_sb[:], xaT[:, :])
            bB_sb = lpool.tile([RANK + 1, DOUT], F32R, tag="bB")
            nc.sync.dma_start(bB_sb[:], bB[:, :])

            def load_x(xpool, k, m0):
                xt = xpool.tile([128, M_BLK], F32R, name="x", tag="x")
                if no_x_dma:
                    nc.sync.dma_start(xt[:, :4], xT[k * 128:(k + 1) * 128, :4])
                else:
                    nc.sync.dma_start(
                        xt[:], xT[k * 128:(k + 1) * 128, m0:m0 + M_BLK])
                return xt

            import contextlib
            rep_ctx = tc.For_i(0, reps, 1) if reps > 1 else contextlib.nullcontext()
            with rep_ctx:
              for blk in range(N_MBLK):
                  m0 = blk * M_BLK
                  xtiles = [None] * KT
                  if not x_in_first_o:
                      for k in range(KT):
                          xtiles[k] = load_x(xpool, k, m0)
                  for o in range(OT):
                      o0 = o * 512
                      psums = [ppool.tile([128, 512], F32, name="ps", tag="ps")
                               for _ in range(MT_PER_BLK)]
                      for k in range(KT):
                          if xtiles[k] is None:
                              xtiles[k] = load_x(xpool, k, m0)
                          wt = wpool.tile([128, 512], F32R, name="w", tag="w")
                          if no_wt_dma:
                              nc.sync.dma_start(
                                  wt[:, :4], wT[k * 128:(k + 1) * 128, :4])
                          else:
                              nc.sync.dma_start(
                                  wt[:], wT[k * 128:(k + 1) * 128, o0:o0 + 512])
                          for mt in range(MT_PER_BLK):
                              nc.tensor.matmul(
                                  psums[mt][:],
                                  xtiles[k][:, mt * 128:(mt + 1) * 128],
                                  wt[:],
                                  start=(k == 0), stop=(no_lora and k == KT - 1))
                      for mt in range(MT_PER_BLK):
                          ms = m0 + mt * 128
                          if not no_lora:
                              nc.tensor.matmul(
                                  psums[mt][:],
                                  xa_sb[:, ms:ms + 128],
                                  bB_sb[:, o0:o0 + 512],
                                  start=False, stop=True)
                          ot = opool.tile([128, 512], F32, name="o", tag="o")
                          nc.vector.tensor_copy(ot[:], psums[mt][:])
                          if not no_out:
                              nc.sync.dma_start(
                                  out[ms:ms + 128, o0:o0 + 512], ot[:])
    nc.compile()
    return nc


def _build_b(no_x_dma=False, no_wt_dma=False, no_lora=False, no_out=False,
             wt_bufs=6, out_bufs=6, x_bufs=KT + 2, reps=1):
    """Orientation B: W slices stationary, resident x moving, out computed
    transposed ([DOUT, SEQ]) and transposed back on host."""
    nc = bacc.Bacc("TRN2", target_bir_lowering=False, debug=False)
    xT = nc.dram_tensor("xT", [DIN, SEQ], F32R, kind="ExternalInput")
    wT = nc.dram_tensor("wT", [DIN, DOUT], F32R, kind="ExternalInput")
    xaT = nc.dram_tensor("xaT", [RANK + 1, SEQ], F32R, kind="ExternalInput")
    bB = nc.dram_tensor("bB", [RANK + 1, DOUT], F32R, kind="ExternalInput")
    outT = nc.dram_tensor("outT", [DOUT, SEQ], F32, kind="ExternalOutput")

    with tile.TileContext(nc) as tc:
        with (
            tc.tile_pool(name="xblk", bufs=x_bufs) as xpool,
            tc.tile_pool(name="wt", bufs=wt_bufs) as wpool,
            tc.tile_pool(name="lora", bufs=1) as lpool,
            tc.tile_pool(name="outp", bufs=out_bufs) as opool,
            tc.tile_pool(name="psum", bufs=8, space="PSUM") as ppool,
        ):
            xa_sb = lpool.tile([RANK + 1, SEQ], F32R, tag="xa")
            nc.sync.dma_start(xa_sb[:], xaT[:, :])
            bB_sb = lpool.tile([RANK + 1, DOUT], F32R, tag="bB")
            nc.sync.dma_start(bB_sb[:], bB[:, :])

            def load_x(k, m0):
                xt = xpool.tile([128, M_BLK], F32R, name="x", tag="x")
                nc.sync.dma_start(
                    xt[:], xT[k * 128:(k + 1) * 128, m0:m0 + M_BLK])
                return xt

            import contextlib
            rep_ctx = tc.For_i(0, reps, 1) if reps > 1 else contextlib.nullcontext()
            with rep_ctx:
              for half in range(N_MBLK):
                m0 = half * M_BLK
                xtiles = [None] * KT
                for og in range(OT):
                    og0 = og * 512
                    psums = [ppool.tile([128, 512], F32, name="ps", tag="ps")
                             for _ in range(8)]
                    for k in range(KT):
                        if xtiles[k] is None:
                            xtiles[k] = load_x(k, m0)
                        wt = wpool.tile([128, 512], F32R, name="w", tag="w")
                        nc.sync.dma_start(
                            wt[:], wT[k * 128:(k + 1) * 128, og0:og0 + 512])
                        for oi in range(4):
                            for mc in range(2):
                                nc.tensor.matmul(
                                    psums[oi * 2 + mc][:],
                                    wt[:, oi * 128:(oi + 1) * 128],
                                    xtiles[k][:, mc * 512:(mc + 1) * 512],
                                    start=(k == 0), stop=False)
                    for oi in range(4):
                        for mc in range(2):
                            nc.tensor.matmul(
                                psums[oi * 2 + mc][:],
                                bB_sb[:, og0 + oi * 128:og0 + (oi + 1) * 128],
                                xa_sb[:, m0 + mc * 512:m0 + (mc + 1) * 512],
                                start=False, stop=True)
                            ot = opool.tile([128, 512], F32, name="o", tag="o")
                            nc.vector.tensor_copy(ot[:], psums[oi * 2 + mc][:])
                            nc.sync.dma_start(
                                outT[og0 + oi * 128:og0 + (oi + 1) * 128,
                                     m0 + mc * 512:m0 + (mc + 1) * 512],
                                ot[:])
    nc.compile()
    return nc


BF16 = mybir.dt.bfloat16
OG = 16                  # out-col groups of 256 (variant C)


def _build_c():
    """Variant C: single fused GEMM out.T = W'.T @ x.T in bf16.

    Host folds A@B into W (W' = W.T + A@B) and pre-tiles W' so each
    out-row group of 256 streams as one contiguous [128, 8192] DMA.
    All 32 x k-tiles ([128, 2048] bf16) stay resident in SBUF; W is
    read exactly once. Bias is added during the PSUM->SBUF drain.
    """
    nc = bacc.Bacc("TRN2", target_bir_lowering=False, debug=False)
    xT = nc.dram_tensor("xT", [DIN, SEQ], BF16, kind="ExternalInput")
    wR = nc.dram_tensor("wR", [OG * 128, KT * 256], BF16, kind="ExternalInput")
    biasR = nc.dram_tensor("biasR", [128, 2 * OG], F32, kind="ExternalInput")
    outT = nc.dram_tensor("outT", [DOUT, SEQ], F32, kind="ExternalOutput")

    with tile.TileContext(nc) as tc:
        with (
            tc.tile_pool(name="xblk", bufs=KT) as xpool,
            tc.tile_pool(name="wt", bufs=8) as wpool,
            tc.tile_pool(name="w0", bufs=KT // 2) as w0pool,
            tc.tile_pool(name="bias", bufs=1) as lpool,
            tc.tile_pool(name="outp", bufs=6) as opool,
            tc.tile_pool(name="psum", bufs=8, space="PSUM") as ppool,
        ):
            bias_sb = lpool.tile([128, 2 * OG], F32, tag="bias")
            nc.sync.dma_start(bias_sb[:], biasR[:, :])

            KC = KT // 4                 # k-steps per W chunk
            CW = KC * 256                # chunk width in wR cols
            xtiles = [None] * KT

            def load_chunk(og, c):
                wc = wpool.tile([128, CW], BF16, name="w", tag="w")
                nc.sync.dma_start(
                    wc[:], wR[og * 128:(og + 1) * 128, c * CW:(c + 1) * CW])
                return wc

            def drain(og, oi, mc, ps):
                ot = opool.tile([128, 512], F32, name="o", tag="o")
                nc.vector.tensor_scalar_add(
                    ot[:], ps[:], bias_sb[:, og * 2 + oi:og * 2 + oi + 1])
                nc.sync.dma_start(
                    outT[og * 256 + oi * 128:og * 256 + (oi + 1) * 128,
                         mc * 512:(mc + 1) * 512],
                    ot[:])

            # 32 groups of (og, oi); each accumulates in 4 PSUM banks so
            # group g+1's matmuls overlap group g's drains (other banks).
            # og 0 streams W in [128,512] mini-chunks interleaved 1:1 with
            # the x tiles so the PE starts ~3us earlier.
            for og in range(OG):
                first = og == 0
                if first:
                    wminis = [None] * (KT // 2)

                    def st_first(k, oi):
                        m = k // 2
                        if wminis[m] is None:
                            wm = w0pool.tile([128, 512], BF16,
                                             name="wm", tag="wm")
                            nc.sync.dma_start(
                                wm[:], wR[og * 128:(og + 1) * 128,
                                          m * 512:(m + 1) * 512])
                            wminis[m] = wm
                        kc = k % 2
                        return wminis[m][:, kc * 256 + oi * 128:
                                         kc * 256 + (oi + 1) * 128]
                else:
                    wchunks = [load_chunk(og, c) for c in range(4)]
                for oi in range(2):
                    last_group = og == OG - 1 and oi == 1
                    psums = [ppool.tile([128, 512], F32, name="ps", tag="ps")
                             for _ in range(4)]
                    if last_group:
                        # mc-outer so each bank finishes its k-sweep early
                        # and drains overlap the remaining banks' matmuls
                        for mc in range(4):
                            for k in range(KT):
                                wsb = wchunks[k // KC]
                                kc = k % KC
                                st = wsb[:, kc * 256 + oi * 128:
                                         kc * 256 + (oi + 1) * 128]
                                nc.tensor.matmul(
                                    psums[mc][:],
                                    st,
                                    xtiles[k][:, mc * 512:(mc + 1) * 512],
                                    start=(k == 0), stop=(k == KT - 1))
                            drain(og, oi, mc, psums[mc])
                        continue
                    for k in range(KT):
                        if first and xtiles[k] is None:
                            xt = xpool.tile([128, SEQ], BF16, name="x", tag="x")
                            nc.sync.dma_start(
                                xt[:], xT[k * 128:(k + 1) * 128, :])
                            xtiles[k] = xt
                        if first:
                            st = st_first(k, oi)
                        else:
                            wsb = wchunks[k // KC]
                            kc = k % KC
                            st = wsb[:, kc * 256 + oi * 128:
                                     kc * 256 + (oi + 1) * 128]
                        for mc in range(4):
                            nc.tensor.matmul(
                                psums[mc][:],
                                st,
                                xtiles[k][:, mc * 512:(mc + 1) * 512],
                                start=(k == 0), stop=(k == KT - 1))
                    for mc in range(4):
                        drain(og, oi, mc, psums[mc])
    nc.compile()
    return nc


VARIANT = "C"


def kernel(x, A, B, weight, bias):
    if not _nc_cache:
        _nc_cache.append(
            {"A": _build, "B": _build_b, "C": _build_c}[VARIANT]())
    nc = _nc_cache[0]

    x = np.asarray(x, dtype=np.float32)
    A = np.asarray(A, dtype=np.float32)
    B = np.asarray(B, dtype=np.float32)
    weight = np.asarray(weight, dtype=np.float32)
    bias = np.asarray(bias, dtype=np.float32)

    if VARIANT == "C":
        from ml_dtypes import bfloat16
        wTf = weight.T + A @ B                               # [DIN, DOUT]
        wR = np.ascontiguousarray(
            wTf.reshape(KT, 128, OG, 256).transpose(2, 1, 0, 3)
               .reshape(OG * 128, KT * 256)).astype(bfloat16)
        biasR = np.ascontiguousarray(
            bias.reshape(2 * OG, 128).T.astype(np.float32))
        in_maps = []
        for b in range(N_CORES):
            xTb = np.ascontiguousarray(x[b].T).astype(bfloat16)
            in_maps.append({"xT": xTb, "wR": wR, "biasR": biasR})
        res = run_bass_kernel_spmd(nc, in_maps, core_ids=list(range(N_CORES)))
        last_result.clear()
        last_result.append(res)
        return np.stack(
            [np.ascontiguousarray(r["outT"].T) for r in res.results], axis=0)

    wT = np.ascontiguousarray(weight.T)                      # [DIN, DOUT]
    bB = np.concatenate([B, bias[None, :]], axis=0)          # [RANK+1, DOUT]
    bB = np.ascontiguousarray(bB, dtype=np.float32)

    in_maps = []
    for b in range(N_CORES):
        xb = x[b]                                            # [SEQ, DIN]
        xTb = np.ascontiguousarray(xb.T)                     # [DIN, SEQ]
        xa = xb @ A                                          # [SEQ, RANK]
        xaT = np.concatenate(
            [np.ascontiguousarray(xa.T),
             np.ones((1, SEQ), dtype=np.float32)], axis=0)   # [RANK+1, SEQ]
        in_maps.append({"xT": xTb, "wT": wT, "xaT": xaT, "bB": bB})

    res = run_bass_kernel_spmd(nc, in_maps, core_ids=list(range(N_CORES)))
    last_result.clear()
    last_result.append(res)
    if VARIANT == "B":
        return np.stack(
            [np.ascontiguousarray(r["outT"].T) for r in res.results], axis=0)
    return np.stack([r["out"] for r in res.results], axis=0)


last_result = []



# revision 15
# speedup vs baseline: 1.1471x; 1.0015x over previous
"""LoRA layer (x @ W.T + (x@A)@B + bias) on 8 trn2 NeuronCores.

Data-parallel: core b computes batch b's (2048, 4096) output slice.
Per-core device work is a single fused matmul: the (4096-deep) base
projection accumulates 32 K=128 fp32r steps into PSUM, then one extra
K=17 step adds the low-rank correction and bias ((x@A | 1) @ (B ; bias)).
x@A (rank 16, 0.4% of FLOPs) and all transposes are host-side prep.
"""
import numpy as np

import concourse.mybir as mybir
import concourse.tile as tile
from concourse import bacc
from concourse.bass_utils import run_bass_kernel_spmd

BATCH, SEQ, DIN, DOUT, RANK = 8, 2048, 4096, 4096, 16
N_CORES = 8

KT = DIN // 128          # 32 contraction tiles
M_BLK = 1024             # tokens resident per block
N_MBLK = SEQ // M_BLK    # 2 blocks
MT_PER_BLK = M_BLK // 128  # 8 m-tiles -> 8 PSUM banks
OT = DOUT // 512         # 8 output-column tiles
F32R = mybir.dt.float32r
F32 = mybir.dt.float32

_nc_cache = []


def _build(no_x_dma=False, no_wt_dma=False, no_lora=False, no_out=False,
           wt_bufs=6, out_bufs=6, x_bufs=KT + 2, x_in_first_o=True, reps=1):
    nc = bacc.Bacc("TRN2", target_bir_lowering=False, debug=False)
    xT = nc.dram_tensor("xT", [DIN, SEQ], F32R, kind="ExternalInput")
    wT = nc.dram_tensor("wT", [DIN, DOUT], F32R, kind="ExternalInput")
    xaT = nc.dram_tensor("xaT", [RANK + 1, SEQ], F32R, kind="ExternalInput")
    bB = nc.dram_tensor("bB", [RANK + 1, DOUT], F32R, kind="ExternalInput")
    out = nc.dram_tensor("out", [SEQ, DOUT], F32, kind="ExternalOutput")

    with tile.TileContext(nc) as tc:
        with (
            tc.tile_pool(name="xblk", bufs=x_bufs) as xpool,
            tc.tile_pool(name="wt", bufs=wt_bufs) as wpool,
            tc.tile_pool(name="lora", bufs=1) as lpool,
            tc.tile_pool(name="outp", bufs=out_bufs) as opool,
            tc.tile_pool(name="psum", bufs=8, space="PSUM") as ppool,
        ):
            xa_sb = lpool.tile([RANK + 1, SEQ], F32R, tag="xa")
            nc.sync.dma_start(xa_sb[:], xaT[:, :])
            bB_sb = lpool.tile([RANK + 1, DOUT], F32R, tag="bB")
            nc.sync.dma_start(bB_sb[:], bB[:, :])

            def load_x(xpool, k, m0):
                xt = xpool.tile([128, M_BLK], F32R, name="x", tag="x")
                if no_x_dma:
                    nc.sync.dma_start(xt[:, :4], xT[k * 128:(k + 1) * 128, :4])
                else:
                    nc.sync.dma_start(
                        xt[:], xT[k * 128:(k + 1) * 128, m0:m0 + M_BLK])
                return xt

            import contextlib
            rep_ctx = tc.For_i(0, reps, 1) if reps > 1 else contextlib.nullcontext()
            with rep_ctx:
              for blk in range(N_MBLK):
                  m0 = blk * M_BLK
                  xtiles = [None] * KT
                  if not x_in_first_o:
                      for k in range(KT):
                          xtiles[k] = load_x(xpool, k, m0)
                  for o in range(OT):
                      o0 = o * 512
                      psums = [ppool.tile([128, 512], F32, name="ps", tag="ps")
                               for _ in range(MT_PER_BLK)]
                      for k in range(KT):
                          if xtiles[k] is None:
                              xtiles[k] = load_x(xpool, k, m0)
                          wt = wpool.tile([128, 512], F32R, name="w", tag="w")
                          if no_wt_dma:
                              nc.sync.dma_start(
                                  wt[:, :4], wT[k * 128:(k + 1) * 128, :4])
                          else:
                              nc.sync.dma_start(
                                  wt[:], wT[k * 128:(k + 1) * 128, o0:o0 + 512])
                          for mt in range(MT_PER_BLK):
                              nc.tensor.matmul(
                                  psums[mt][:],
                                  xtiles[k][:, mt * 128:(mt + 1) * 128],
                                  wt[:],
                                  start=(k == 0), stop=(no_lora and k == KT - 1))
                      for mt in range(MT_PER_BLK):
                          ms = m0 + mt * 128
                          if not no_lora:
                              nc.tensor.matmul(
                                  psums[mt][:],
                                  xa_sb[:, ms:ms + 128],
                                  bB_sb[:, o0:o0 + 512],
                                  start=False, stop=True)
                          ot = opool.tile([128, 512], F32, name="o", tag="o")
                          nc.vector.tensor_copy(ot[:], psums[mt][:])
                          if not no_out:
                              nc.sync.dma_start(
                                  out[ms:ms + 128, o0:o0 + 512], ot[:])
    nc.compile()
    return nc


def _build_b(no_x_dma=False, no_wt_dma=False, no_lora=False, no_out=False,
             wt_bufs=6, out_bufs=6, x_bufs=KT + 2, reps=1):
    """Orientation B: W slices stationary, resident x moving, out computed
    transposed ([DOUT, SEQ]) and transposed back on host."""
    nc = bacc.Bacc("TRN2", target_bir_lowering=False, debug=False)
    xT = nc.dram_tensor("xT", [DIN, SEQ], F32R, kind="ExternalInput")
    wT = nc.dram_tensor("wT", [DIN, DOUT], F32R, kind="ExternalInput")
    xaT = nc.dram_tensor("xaT", [RANK + 1, SEQ], F32R, kind="ExternalInput")
    bB = nc.dram_tensor("bB", [RANK + 1, DOUT], F32R, kind="ExternalInput")
    outT = nc.dram_tensor("outT", [DOUT, SEQ], F32, kind="ExternalOutput")

    with tile.TileContext(nc) as tc:
        with (
            tc.tile_pool(name="xblk", bufs=x_bufs) as xpool,
            tc.tile_pool(name="wt", bufs=wt_bufs) as wpool,
            tc.tile_pool(name="lora", bufs=1) as lpool,
            tc.tile_pool(name="outp", bufs=out_bufs) as opool,
            tc.tile_pool(name="psum", bufs=8, space="PSUM") as ppool,
        ):
            xa_sb = lpool.tile([RANK + 1, SEQ], F32R, tag="xa")
            nc.sync.dma_start(xa_sb[:], xaT[:, :])
            bB_sb = lpool.tile([RANK + 1, DOUT], F32R, tag="bB")
            nc.sync.dma_start(bB_sb[:], bB[:, :])

            def load_x(k, m0):
                xt = xpool.tile([128, M_BLK], F32R, name="x", tag="x")
                nc.sync.dma_start(
                    xt[:], xT[k * 128:(k + 1) * 128, m0:m0 + M_BLK])
                return xt

            import contextlib
            rep_ctx = tc.For_i(0, reps, 1) if reps > 1 else contextlib.nullcontext()
            with rep_ctx:
              for half in range(N_MBLK):
                m0 = half * M_BLK
                xtiles = [None] * KT
                for og in range(OT):
                    og0 = og * 512
                    psums = [ppool.tile([128, 512], F32, name="ps", tag="ps")
                             for _ in range(8)]
                    for k in range(KT):
                        if xtiles[k] is None:
                            xtiles[k] = load_x(k, m0)
                        wt = wpool.tile([128, 512], F32R, name="w", tag="w")
                        nc.sync.dma_start(
                            wt[:], wT[k * 128:(k + 1) * 128, og0:og0 + 512])
                        for oi in range(4):
                            for mc in range(2):
                                nc.tensor.matmul(
                                    psums[oi * 2 + mc][:],
                                    wt[:, oi * 128:(oi + 1) * 128],
                                    xtiles[k][:, mc * 512:(mc + 1) * 512],
                                    start=(k == 0), stop=False)
                    for oi in range(4):
                        for mc in range(2):
                            nc.tensor.matmul(
                                psums[oi * 2 + mc][:],
                                bB_sb[:, og0 + oi * 128:og0 + (oi + 1) * 128],
                                xa_sb[:, m0 + mc * 512:m0 + (mc + 1) * 512],
                                start=False, stop=True)
                            ot = opool.tile([128, 512], F32, name="o", tag="o")
                            nc.vector.tensor_copy(ot[:], psums[oi * 2 + mc][:])
                            nc.sync.dma_start(
                                outT[og0 + oi * 128:og0 + (oi + 1) * 128,
                                     m0 + mc * 512:m0 + (mc + 1) * 512],
                                ot[:])
    nc.compile()
    return nc


BF16 = mybir.dt.bfloat16
OG = 16                  # out-col groups of 256 (variant C)


def _build_c():
    """Variant C: single fused GEMM out.T = W'.T @ x.T in bf16.

    Host folds A@B into W (W' = W.T + A@B) and pre-tiles W' so each
    out-row group of 256 streams as one contiguous [128, 8192] DMA.
    All 32 x k-tiles ([128, 2048] bf16) stay resident in SBUF; W is
    read exactly once. Bias is added during the PSUM->SBUF drain.
    """
    nc = bacc.Bacc("TRN2", target_bir_lowering=False, debug=False)
    xT = nc.dram_tensor("xT", [DIN, SEQ], BF16, kind="ExternalInput")
    wR = nc.dram_tensor("wR", [OG * 128, KT * 256], BF16, kind="ExternalInput")
    biasR = nc.dram_tensor("biasR", [128, 2 * OG], F32, kind="ExternalInput")
    outT = nc.dram_tensor("outT", [DOUT, SEQ], F32, kind="ExternalOutput")

    with tile.TileContext(nc) as tc:
        with (
            tc.tile_pool(name="xblk", bufs=KT) as xpool,
            tc.tile_pool(name="wt", bufs=8) as wpool,
            tc.tile_pool(name="w0", bufs=KT // 2) as w0pool,
            tc.tile_pool(name="x0", bufs=4) as x0pool,
            tc.tile_pool(name="bias", bufs=1) as lpool,
            tc.tile_pool(name="outp", bufs=6) as opool,
            tc.tile_pool(name="psum", bufs=8, space="PSUM") as ppool,
        ):
            bias_sb = lpool.tile([128, 2 * OG], F32, tag="bias")
            bias_pending = [True]

            def load_bias():
                if bias_pending[0]:
                    nc.sync.dma_start(bias_sb[:], biasR[:, :])
                    bias_pending[0] = False

            # Warm the PE HAM clock gate with tiny dummy matmuls while the
            # first DMAs are in flight, so real matmuls start at 2.4 GHz.
            warm_sb = lpool.tile([128, 8], F32, tag="warm")
            nc.vector.memset(warm_sb[:], 1.0)
            warm_ps = ppool.tile([128, 512], F32, name="wps", tag="ps")
            for _ in range(30):
                nc.tensor.matmul(warm_ps[:1, :8], warm_sb[:, 0:1],
                                 warm_sb[:, :8], start=True, stop=True)

            KC = KT // 4                 # k-steps per W chunk
            CW = KC * 256                # chunk width in wR cols
            xtiles = [None] * KT

            def load_chunk(og, c):
                wc = wpool.tile([128, CW], BF16, name="w", tag="w")
                nc.sync.dma_start(
                    wc[:], wR[og * 128:(og + 1) * 128, c * CW:(c + 1) * CW])
                return wc

            def xslice(k, mc):
                t = xtiles[k]
                if isinstance(t, list):
                    return t[mc][:]
                return t[:, mc * 512:(mc + 1) * 512]

            def drain(og, oi, mc, ps):
                ot = opool.tile([128, 512], F32, name="o", tag="o")
                nc.vector.tensor_scalar_add(
                    ot[:], ps[:], bias_sb[:, og * 2 + oi:og * 2 + oi + 1])
                nc.sync.dma_start(
                    outT[og * 256 + oi * 128:og * 256 + (oi + 1) * 128,
                         mc * 512:(mc + 1) * 512],
                    ot[:])

            # 32 groups of (og, oi); each accumulates in 4 PSUM banks so
            # group g+1's matmuls overlap group g's drains (other banks).
            # og 0 streams W in [128,512] mini-chunks interleaved 1:1 with
            # the x tiles so the PE starts ~3us earlier.
            for og in range(OG):
                first = og == 0
                if first:
                    wminis = [None] * (KT // 2)

                    def st_first(k, oi):
                        m = k // 2
                        if wminis[m] is None:
                            wm = w0pool.tile([128, 512], BF16,
                                             name="wm", tag="wm")
                            nc.sync.dma_start(
                                wm[:], wR[og * 128:(og + 1) * 128,
                                          m * 512:(m + 1) * 512])
                            wminis[m] = wm
                        kc = k % 2
                        return wminis[m][:, kc * 256 + oi * 128:
                                         kc * 256 + (oi + 1) * 128]
                else:
                    wchunks = [load_chunk(og, c) for c in range(4)]
                for oi in range(2):
                    last_group = og == OG - 1 and oi == 1
                    psums = [ppool.tile([128, 512], F32, name="ps", tag="ps")
                             for _ in range(4)]
                    if last_group:
                        # mc-outer so each bank finishes its k-sweep early
                        # and drains overlap the remaining banks' matmuls
                        for mc in range(4):
                            for k in range(KT):
                                wsb = wchunks[k // KC]
                                kc = k % KC
                                st = wsb[:, kc * 256 + oi * 128:
                                         kc * 256 + (oi + 1) * 128]
                                nc.tensor.matmul(
                                    psums[mc][:],
                                    st,
                                    xslice(k, mc),
                                    start=(k == 0), stop=(k == KT - 1))
                            drain(og, oi, mc, psums[mc])
                        continue
                    for k in range(KT):
                        if first and xtiles[k] is None:
                            if k == 0:
                                # split k=0 so the first matmul's moving
                                # operand lands as a small early DMA
                                parts = []
                                for mc in range(4):
                                    xp = x0pool.tile([128, 512], BF16,
                                                     name="xp", tag="xp")
                                    nc.sync.dma_start(
                                        xp[:], xT[0:128,
                                                  mc * 512:(mc + 1) * 512])
                                    parts.append(xp)
                                xtiles[0] = parts
                            else:
                                xt = xpool.tile([128, SEQ], BF16,
                                                name="x", tag="x")
                                nc.sync.dma_start(
                                    xt[:], xT[k * 128:(k + 1) * 128, :])
                                xtiles[k] = xt
                        if first and k == 1 and oi == 0:
                            load_bias()
                        if first:
                            st = st_first(k, oi)
                        else:
                            wsb = wchunks[k // KC]
                            kc = k % KC
                            st = wsb[:, kc * 256 + oi * 128:
                                     kc * 256 + (oi + 1) * 128]
                        for mc in range(4):
                            nc.tensor.matmul(
                                psums[mc][:],
                                st,
                                xslice(k, mc),
                                start=(k == 0), stop=(k == KT - 1))
                    for mc in range(4):
                        drain(og, oi, mc, psums[mc])
    nc.compile()
    return nc


VARIANT = "C"


def kernel(x, A, B, weight, bias):
    if not _nc_cache:
        _nc_cache.append(
            {"A": _build, "B": _build_b, "C": _build_c}[VARIANT]())
    nc = _nc_cache[0]

    x = np.asarray(x, dtype=np.float32)
    A = np.asarray(A, dtype=np.float32)
    B = np.asarray(B, dtype=np.float32)
    weight = np.asarray(weight, dtype=np.float32)
    bias = np.asarray(bias, dtype=np.float32)

    if VARIANT == "C":
        from ml_dtypes import bfloat16
        wTf = weight.T + A @ B                               # [DIN, DOUT]
        wR = np.ascontiguousarray(
            wTf.reshape(KT, 128, OG, 256).transpose(2, 1, 0, 3)
               .reshape(OG * 128, KT * 256)).astype(bfloat16)
        biasR = np.ascontiguousarray(
            bias.reshape(2 * OG, 128).T.astype(np.float32))
        in_maps = []
        for b in range(N_CORES):
            xTb = np.ascontiguousarray(x[b].T).astype(bfloat16)
            in_maps.append({"xT": xTb, "wR": wR, "biasR": biasR})
        res = run_bass_kernel_spmd(nc, in_maps, core_ids=list(range(N_CORES)))
        last_result.clear()
        last_result.append(res)
        return np.stack(
            [np.ascontiguousarray(r["outT"].T) for r in res.results], axis=0)

    wT = np.ascontiguousarray(weight.T)                      # [DIN, DOUT]
    bB = np.concatenate([B, bias[None, :]], axis=0)          # [RANK+1, DOUT]
    bB = np.ascontiguousarray(bB, dtype=np.float32)

    in_maps = []
    for b in range(N_CORES):
        xb = x[b]                                            # [SEQ, DIN]
        xTb = np.ascontiguousarray(xb.T)                     # [DIN, SEQ]
        xa = xb @ A                                          # [SEQ, RANK]
        xaT = np.concatenate(
            [np.ascontiguousarray(xa.T),
             np.ones((1, SEQ), dtype=np.float32)], axis=0)   # [RANK+1, SEQ]
        in_maps.append({"xT": xTb, "wT": wT, "xaT": xaT, "bB": bB})

    res = run_bass_kernel_spmd(nc, in_maps, core_ids=list(range(N_CORES)))
    last_result.clear()
    last_result.append(res)
    if VARIANT == "B":
        return np.stack(
            [np.ascontiguousarray(r["outT"].T) for r in res.results], axis=0)
    return np.stack([r["out"] for r in res.results], axis=0)


last_result = []

